# revision 48
# baseline (speedup 1.0000x reference)
"""7x7 median blur (kornia median_blur semantics: zero padding, lower
median) for img [8, 3, 512, 512] f32, data-parallel across 8 NeuronCores
(one batch image per core).

Design v3 (QUAD=True, the active path): quad-decomposed selection
network.  Every 7-wide window [x-3,x+3] = one 4-aligned quad of sorted
columns (28-merge, amortized over 4 output columns) + one 2-aligned
pair (14-merge, amortized over 2) + 1 single sorted column.  Outputs
x=4q+{0,1} share band A = merge(Q[q], P2[q-1]); x=4q+{2,3} share
band B = merge(Q[q], P0[q+1]); rank-pruning drops quad ranks 1-3 and
26-28 before the band (target rank 25 -> 22).  All compute planes are
column-parity-packed (one value per 4 image columns) so every
TensorTensor keeps a stride-1 free dim and the DVE 2x bf16 mode; lane
packing ([r2|r0|r3|r1] col regions, [P2|P0] pairs, [bandA|bandB])
turns cross-variant op pairs into single S3S3D3-legal instructions.
The column sorts read the raw DMA-staged rows directly at stride 4
(per-variant ops), eliminating a separate packing pass; sorted-col
edge slots are re-zeroed by memsets to preserve the zero-padding
semantics.  ~302 plane ops/group vs 238 full-width ops in v2, but at
~55% of the element traffic: measured 489us/core vs 815us for v2
(same bf16 datapath, rel err 2.0e-3 = pure bf16 rounding).

Design (v2), kept below for reference (QUAD=False):

- Exact min/max selection network (238 plane ops): sort the 7 vertical
  taps per column (16-CE optimal sort), build shared 2-col and 3-col
  odd-even merges, merge two shifted 3-col lists into the 42-element
  union pruned to ranks 18..25, then select rank 25 against the 7th
  column via the min-over-splits-of-max identity. Horizontal window
  shifts are free AP offsets; vertical taps are DMA-shifted row copies.
- bf16 datapath: rounding is monotone, and monotone maps commute with
  order statistics, so the device result is exactly bf16(median(img));
  rel err <= 2^-8 (measured 2.0e-3) against the f32 reference. bf16
  halves SBUF bytes (G=6 row-blocks per instruction) and runs the DVE
  in its 2x perf mode.
- Flat pitch-524 layout: each (channel, row-block) occupies a 524-col
  segment (518 valid + 6 dead). Every network op is ONE contiguous
  free-dim row covering all G=6 blocks; the <=6-col AP shifts never
  cross into the next block's valid data, and garbage written outside a
  node's backward-computed needed interval is never read.
- Arena allocation: the 7 tap planes die after the column-sort phase, so
  taps and network slots share one 30-region arena (189KB/partition).
  G=6 -> 2 groups x 238 TT ops = 476 DVE instructions per core.
- Wait hygiene: this toolchain allows ONE sync wait per instruction;
  _reduce_waits drops waits implied transitively (for the ISA limit) and
  _strip_waits additionally drops every wait already implied by in-order
  queue execution (sems only increase, so such waits are runtime no-ops).
- Cached executable: the stock run_bass_kernel_spmd path rebuilds the
  XLA executable every call (full retrace + compile with the BIR inlined
  in the HLO). _get_runner compiles once per program and reuses the
  jitted callable, leaving only dispatch + device execution per call.

Measured on this setup: device exec ~0.85ms per core-program (vs 1.80ms
for the previous f32 G=2 kernel measured the same way), matching the
DVE model 476 ops x (3144/2 + 151) cycles @ 0.96 GHz. DMA fully
overlaps. Correctness: max rel err 2.0e-3 (pure bf16 rounding).
"""

import numpy as np

H = 512
W = 512
C = 3
B = 8
K = 7
PAD = K // 2
WT = W + 2 * PAD      # valid tile width per block (518)
PITCH = WT + 2 * PAD  # block pitch in flat layout (524); 6 dead cols
NUNITS = C * (H // 128)  # 12 (channel, row-block) units per core

G = 6
DTYPE = "bf16"


# ---------------------------------------------------------------- network ---

def _build_network():
    """Median-of-49 min/max selection network in SSA form.

    Returns (emitted, final_id):
      emitted: list of ('min'|'max', ssa_id, (a_id, da), (b_id, db), lo, hi)
        in dependency order; ids < 0 encode taps (id -1-k = vertical tap k);
        [lo, hi] inclusive is the needed write interval in tile idx space
        (0..517), tile idx i = image column i-3.
    """
    ops = []
    cache = {}

    def inp(k):
        key = ("in", k)
        if key not in cache:
            ops.append(key)
            cache[key] = len(ops) - 1
        return (cache[key], 0)

    def mk(op, a, b):
        (ia, da), (ib, db) = a, b
        if (ia, da) > (ib, db):
            (ia, da), (ib, db) = (ib, db), (ia, da)
        base = min(da, db)
        key = (op, ia, da - base, ib, db - base)
        if key not in cache:
            ops.append((op, (ia, da - base), (ib, db - base)))
            cache[key] = len(ops) - 1
        return (cache[key], base)

    def ce(a, b):
        return mk("min", a, b), mk("max", a, b)

    def oe_merge(A, Bl):
        n, m = len(A), len(Bl)
        if n == 0:
            return list(Bl)
        if m == 0:
            return list(A)
        if n == 1 and m == 1:
            lo, hi = ce(A[0], Bl[0])
            return [lo, hi]
        E = oe_merge(A[0::2], Bl[0::2])
        O = oe_merge(A[1::2], Bl[1::2])
        out = [E[0]]
        i = 0
        while i < len(O) and i + 1 < len(E):
            lo, hi = ce(O[i], E[i + 1])
            out += [lo, hi]
            i += 1
        out += O[i:]
        out += E[i + 1:]
        return out

    def shift(ws, dx):
        return [(i, d + dx) for (i, d) in ws]

    def select_rank(A, Bl, r):
        n, m = len(A), len(Bl)
        cands = []
        for i in range(max(0, r - m), min(r, n) + 1):
            j = r - i
            if i == 0:
                cands.append(Bl[j - 1])
            elif j == 0:
                cands.append(A[i - 1])
            else:
                cands.append(mk("max", A[i - 1], Bl[j - 1]))
        while len(cands) > 1:
            nxt = []
            for q in range(0, len(cands) - 1, 2):
                nxt.append(mk("min", cands[q], cands[q + 1]))
            if len(cands) % 2:
                nxt.append(cands[-1])
            cands = nxt
        return cands[0]

    w = [inp(k) for k in range(K)]
    sort_pairs = [(0, 6), (2, 3), (4, 5), (0, 2), (1, 4), (3, 6), (0, 1),
                  (2, 5), (3, 4), (1, 2), (4, 6), (2, 3), (4, 5), (1, 2),
                  (3, 4), (5, 6)]
    for i, j in sort_pairs:
        lo, hi = ce(w[i], w[j])
        w[i], w[j] = lo, hi
    col = w
    m2 = oe_merge(col, shift(col, 1))
    m3 = oe_merge(m2, shift(col, 2))
    n42 = oe_merge(shift(m3, -3), m3)
    out_id, out_dx = select_rank(n42, shift(col, 3), 25)
    assert out_dx == -3

    # dead-code elimination from the output node
    needed = set()
    stack = [out_id]
    while stack:
        i = stack.pop()
        if i in needed:
            continue
        needed.add(i)
        op = ops[i]
        if op[0] != "in":
            stack.append(op[1][0])
            stack.append(op[2][0])
    sched = [i for i in range(len(ops)) if i in needed]

    # forward valid intervals
    valid = {}
    for i in sched:
        op = ops[i]
        if op[0] == "in":
            valid[i] = (0, WT - 1)
        else:
            _, (a, da), (b, db) = op
            lo = max(valid[a][0] - da, valid[b][0] - db, 0)
            hi = min(valid[a][1] - da, valid[b][1] - db, WT - 1)
            valid[i] = (lo, hi)

    # backward needed intervals; output cols 0..511 live at tile idx 0..511
    need_iv = {out_id: (0, W - 1)}
    for i in reversed(sched):
        op = ops[i]
        if op[0] == "in" or i not in need_iv:
            continue
        lo, hi = need_iv[i]
        for (a, da) in (op[1], op[2]):
            nlo, nhi = lo + da, hi + da
            if a in need_iv:
                nlo = min(nlo, need_iv[a][0])
                nhi = max(nhi, need_iv[a][1])
            need_iv[a] = (nlo, nhi)
    for i in sched:
        if ops[i][0] != "in":
            assert need_iv[i][0] >= valid[i][0] and need_iv[i][1] <= valid[i][1]

    # SSA renumber: compute nodes 0..n-1 in sched order, taps negative
    ssa = {}
    nxt = 0
    for i in sched:
        if ops[i][0] == "in":
            ssa[i] = -1 - ops[i][1]
        else:
            ssa[i] = nxt
            nxt += 1
    emitted = []
    for i in sched:
        op = ops[i]
        if op[0] == "in":
            continue
        o, (a, da), (b, db) = op
        lo, hi = need_iv[i]
        emitted.append((o, ssa[i], (ssa[a], da), (ssa[b], db), lo, hi))
    return emitted, ssa[out_id]


def _arena_alloc(emitted, final_id):
    """Linear-scan region allocation over SSA values; taps pre-own regions
    0..6 (freed after the sort phase); the final node lives in a separate
    tile (out-DMA WAR isolation), not the arena."""
    last_use = {}
    for t, (_, s, (sa, _), (sb, _), _, _) in enumerate(emitted):
        for a in (sa, sb):
            last_use[a] = t
    region = {-1 - k: k for k in range(K)}
    free = []
    n_regions = K
    for t, (_, s, (sa, _), (sb, _), _, _) in enumerate(emitted):
        if s == final_id:
            region[s] = None
        elif free:
            region[s] = free.pop()
        else:
            region[s] = n_regions
            n_regions += 1
        for a in {sa, sb}:
            if last_use.get(a) == t and region[a] is not None:
                free.append(region[a])
    return region, n_regions


# ---------------------------------------------------------- quad network ---
# Quad decomposition: every 7-col window = one 4-aligned quad (28-merge,
# built once per 4 columns) + one 2-aligned pair (14-merge, once per 2
# columns) + 1 single sorted column.  Outputs x=4q+{0,1} share
# band42A = merge(Q[q], P2[q-1]); x=4q+{2,3} share band42B =
# merge(Q[q], P0[q+1]).  Median = select_rank(band42, single_col, 25).
# Packed planes: one value per 4 image columns; s-slot s=q+1 in [0,129],
# col c = 4(s-1)+r, slots s=0,129 hold zeros (horizontal padding).
# nv=4 values (taps/col sorts) have lanes [r2|r0|r3|r1] (so the pair
# build is lane-contiguous); nv=2: pairs [P2|P0], bands [A|B]; nv=1:
# quads + select intermediates.  Element work is ~55% of the v2 network.

SP_Q = 130          # sub-plane pitch in s-slots
NS_Q = 130          # used slots: s in [0, 129]
NQ_Q = 128          # valid q positions (q in [0,127])
SUB_OF_R = {2: 0, 0: 1, 3: 2, 1: 3}


def _build_network_quad():
    """Emit the quad network.  Op: (alu, d, aref, bref, lo, hi) with
    ref=(vid, sub0, dsub, q0, dq): lane l reads value vid at sub-plane
    sub0+l*dsub shifted by q0+l*dq; dst lanes 0..nv-1 written on
    s in [lo,hi].  Returns (values: vid->nv, taps, emitted, finals)."""
    values = {}
    ops = []
    cache = {}
    finals = {}

    def new_val(nv):
        vid = len(values)
        values[vid] = nv
        return vid

    taps = [new_val(4) for _ in range(K)]

    NINF = ("NINF", 0, 0, 0, 0)
    PINF = ("PINF", 0, 0, 0, 0)

    def vref(vid):
        return (vid, 0, 1, 0, 0)

    def mk(alu, nv, aref, bref):
        # sentinel simplification: comparisons against +-inf are free
        for x, y in ((aref, bref), (bref, aref)):
            if x == NINF:
                return x if alu == "min" else y
            if x == PINF:
                return y if alu == "min" else x
        key = (alu, nv, aref, bref)
        if key not in cache:
            d = new_val(nv)
            ops.append((alu, d, aref, bref))
            cache[key] = vref(d)
        return cache[key]

    def ce(nv, aref, bref):
        return mk("min", nv, aref, bref), mk("max", nv, aref, bref)

    def oe_merge(nv, A, B):
        n, m = len(A), len(B)
        if n == 0:
            return list(B)
        if m == 0:
            return list(A)
        if n == 1 and m == 1:
            lo, hi = ce(nv, A[0], B[0])
            return [lo, hi]
        E = oe_merge(nv, A[0::2], B[0::2])
        O = oe_merge(nv, A[1::2], B[1::2])
        out = [E[0]]
        i = 0
        while i < len(O) and i + 1 < len(E):
            lo, hi = ce(nv, O[i], E[i + 1])
            out += [lo, hi]
            i += 1
        out += O[i:]
        out += E[i + 1:]
        return out

    sort_pairs = [(0, 6), (2, 3), (4, 5), (0, 2), (1, 4), (3, 6), (0, 1),
                  (2, 5), (3, 4), (1, 2), (4, 6), (2, 3), (4, 5), (1, 2),
                  (3, 4), (5, 6)]
    perm = (0, 6, 2, 3, 4, 5, 1)   # sorting is input-order invariant
    w = [vref(taps[perm[i]]) for i in range(K)]
    for i, j in sort_pairs:
        lo, hi = ce(4, w[i], w[j])
        w[i], w[j] = lo, hi
    cols = w
    phase_bounds = [len(values)]

    # pairs: P2 = merge(subs 0,2) lanes, P0 = merge(subs 1,3)
    A = [(c[0], 0, 1, 0, 0) for c in cols]
    B = [(c[0], 2, 1, 0, 0) for c in cols]
    P = oe_merge(2, A, B)
    phase_bounds.append(len(values))

    # quads: merge(P2 lane, P0 lane)
    A = [(p[0], 0, 0, 0, 0) for p in P]
    B = [(p[0], 1, 0, 0, 0) for p in P]
    import os as _osq
    if _osq.environ.get('QUAD_QBITONIC', '0') == '1':
        # bitonic-32 quad: +14 ops vs oe but fully staged structure
        L = A + [PINF] * 3 + list(reversed([NINF] + B))
        d = 16
        while d >= 1:
            NL = list(L)
            for i in range(32):
                if (i % (2 * d)) < d and i + d < 32:
                    NL[i] = mk("min", 1, L[i], L[i + d])
                    NL[i + d] = mk("max", 1, L[i], L[i + d])
            L = NL
            d //= 2
        # 1 NINF below: real rank i (1-idx) = L[i]; band needs 4..25
        Q = [None] * 3 + L[4:26]   # align so Q[3:25] = real ranks 4..25
    else:
        Q = oe_merge(1, A, B)
    phase_bounds.append(len(values))

    # bands: lane A = merge(Q[q], P2[q-1]); lane B = merge(Q[q], P0[q+1])
    # rank pruning: Q[i] (1-idx) has global rank in [i, i+21]; only
    # i in [4,25] can be the 25th smallest -> drop Q[1..3]/Q[26..28];
    # the select target over (band36 + col7) becomes rank 22.
    # bitonic band (searched): Q' asc + PINF fill + reversed(P + 2 NINF)
    # is a bitonic sequence of 64; the staged bitonic merger DCE's to 52
    # ops for the 8-rank band vs 62 for the best sentinel-padded
    # odd-even merge.  Target rank shifts by the 2 -inf: 22 -> 24.
    Araw = [(qq[0], 0, 0, 0, 0) for qq in Q[3:25]]
    Braw = [NINF, NINF] + [(p[0], 0, 1, -1, 2) for p in P]
    L = Araw + [PINF] * (64 - len(Araw) - len(Braw)) + list(reversed(Braw))
    d = 32
    while d >= 1:
        NL = list(L)
        for i in range(64):
            if (i % (2 * d)) < d and i + d < 64:
                NL[i] = mk("min", 2, L[i], L[i + d])
                NL[i + d] = mk("max", 2, L[i], L[i + d])
        L = NL
        d //= 2
    M42 = L
    phase_bounds.append(len(values))

    def select_rank2(A, B, r):
        n, m = len(A), len(B)
        cands = []
        for i in range(max(0, r - m), min(r, n) + 1):
            j = r - i
            if i == 0:
                cands.append(B[j - 1])
            elif j == 0:
                cands.append(A[i - 1])
            else:
                cands.append(mk("max", 2, A[i - 1], B[j - 1]))
        cands = [c for c in cands if c[0] not in ("NINF", "PINF")]
        while len(cands) > 1:
            nxt = []
            for q2 in range(0, len(cands) - 1, 2):
                nxt.append(mk("min", 2, cands[q2], cands[q2 + 1]))
            if len(cands) % 2:
                nxt.append(cands[-1])
            cands = nxt
        return cands[0]

    # selects as nv=2 PAIRS: pair p=0 lanes (r0, r2), p=1 lanes (r1, r3):
    # lane l reads band lane l; singles: p=0 cols sub 3-l @ q-1 (r1, r3),
    # p=1 cols sub 1-l @ q+1 (r0, r2).  Final ft lanes interleave at
    # stride 2 (x = 4q + p + 2l).
    Ar = list(M42)
    for pr, (csub0, cq) in {0: (3, -1), 1: (1, +1)}.items():
        Br = [(c[0], csub0, -1, cq, 0) for c in cols]
        finals[pr] = select_rank2(Ar, Br, 24)[0]
    _build_network_quad._phase_bounds = phase_bounds + [len(values)]

    prod = {d: op for op in ops for d in [op[1]]}
    needed = set()
    stack = list(finals.values())
    while stack:
        v = stack.pop()
        if v in needed or v in taps:
            continue
        needed.add(v)
        _, _, aref, bref = prod[v]
        stack.append(aref[0])
        stack.append(bref[0])
    emitted = [op for op in ops if op[1] in needed]

    need_iv = {v: (1, NQ_Q) for v in finals.values()}
    for (alu, d, aref, bref) in reversed(emitted):
        if d not in need_iv:
            continue
        lo, hi = need_iv[d]
        nv = values[d]
        for (sv, sub0, dsub, q0, dq) in (aref, bref):
            sh = [q0 + l * dq for l in range(nv)]
            nlo, nhi = lo + min(sh), hi + max(sh)
            if sv in need_iv:
                nlo = min(nlo, need_iv[sv][0])
                nhi = max(nhi, need_iv[sv][1])
            need_iv[sv] = (max(nlo, 0), min(nhi, NS_Q - 1))
    out = []
    for (alu, d, aref, bref) in emitted:
        lo, hi = need_iv[d]
        assert 0 <= lo <= hi <= NS_Q - 1
        out.append((alu, d, aref, bref, lo, hi))
    col_vids = [c[0] for c in cols]
    # phase id per value (1=sort, 2=pairs, 3=quads, 4=band, 5=selects)
    pb = _build_network_quad._phase_bounds
    phase_of = {}
    for (alu, d, aref, bref) in emitted:
        for ph, bound in enumerate(pb, start=1):
            if d < bound:
                phase_of[d] = ph
                break
    return values, taps, out, finals, col_vids, phase_of


def _arena_alloc_quad(values, taps, emitted, finals, fused_first=None):
    """Linear-scan region allocation per size class (nv in {4,2,1}).
    Taps live in the staging zone (raw rows), not the arena."""
    last_use = {}
    for t, (alu, d, aref, bref, lo, hi) in enumerate(emitted):
        last_use[aref[0]] = t
        last_use[bref[0]] = t
    region = {}
    free = {4: [], 2: [], 1: []}
    nreg = {4: 0, 2: 0, 1: 0}
    fin = set(finals.values())
    deferred = []
    for t, (alu, d, aref, bref, lo, hi) in enumerate(emitted):
        nv = values[d]
        if d in fin:
            region[d] = None
        elif free[nv]:
            region[d] = free[nv].pop()
        else:
            region[d] = nreg[nv]
            nreg[nv] += 1
        if fused_first is not None and t in fused_first:
            # first half of a fused pair: its sources stay blocked until
            # the partner (next record) has allocated its destination
            deferred = [(sv, t) for sv in {aref[0], bref[0]}]
            continue
        for sv, tt in deferred:
            if last_use.get(sv) == tt and region.get(sv) is not None:
                free[values[sv]].append(region[sv])
        deferred = []
        for sv in {aref[0], bref[0]}:
            if last_use.get(sv) == t and region.get(sv) is not None:
                free[values[sv]].append(region[sv])
    return region, nreg


# ----------------------------------------------------------------- kernel ---

_CACHE = {}


def _reduce_waits(nc, mybir):
    """Transitive reduction of multi-wait instructions (this toolchain
    allows at most ONE sync wait per instruction).

    Tile emits each instruction's required vector-clock ticks as waits
    without cross-proc transitivity: if X waits on DVE>=929 and the DVE
    instruction achieving tick 929 itself (transitively) waited on
    DMAHW0>=16, then X's DMAHW0>=16 wait is redundant. Sound because sem
    values only increase."""
    import bisect
    from collections import defaultdict

    f = nc.m.functions[0]
    insts = [ins for blk in f.blocks for ins in blk.instructions]

    cum = defaultdict(int)
    sem_hist = defaultdict(lambda: ([], []))
    bad_sems = set()
    for idx, ins in enumerate(insts):
        si = ins.sync_info
        if not si:
            continue
        for up in (si.on_update or []):
            if getattr(up, "update_mode", None) in ("sem-inc", "sem-add-imm"):
                cum[up.id] += up.update_value
                vals, idxs = sem_hist[up.id]
                vals.append(cum[up.id])
                idxs.append(idx)
            else:
                bad_sems.add(up.id)

    def achiever(sem, v):
        if sem in bad_sems:
            return None
        vals, idxs = sem_hist.get(sem, ([], []))
        i = bisect.bisect_left(vals, v)
        return idxs[i] if i < len(vals) else None

    know = [None] * len(insts)
    last_on_proc = {}
    reducible = ("InstDMACopy", "InstTensorTensor", "InstTensorCopy",
                 "InstMemset")
    still_multi = []
    for idx, ins in enumerate(insts):
        proc = getattr(ins, "bass_scheduled_proc", None)
        base = {}
        if proc is not None and proc in last_on_proc:
            base = dict(know[last_on_proc[proc]])
        si = ins.sync_info
        waits = list(si.on_wait or []) if si else []
        usable = [w for w in waits
                  if getattr(w, "wait_mode", None) == "sem-ge-imm"
                  and w.wait_reg is None and w.id not in bad_sems]
        cur = dict(base)
        kept = list(waits)
        if si and len(waits) > 1 and len(usable) == len(waits):
            wk = []
            for w in waits:
                a = achiever(w.id, w.wait_value)
                k = dict(know[a]) if (a is not None and know[a]) else {}
                k[w.id] = max(k.get(w.id, 0), w.wait_value)
                wk.append(k)
            order = sorted(range(len(waits)), key=lambda i: -len(wk[i]))
            keep_idx = []
            for wi in order:
                w = waits[wi]
                if cur.get(w.id, 0) >= w.wait_value:
                    continue
                keep_idx.append(wi)
                for s, v in wk[wi].items():
                    cur[s] = max(cur.get(s, 0), v)
            kept = [waits[i] for i in sorted(keep_idx)]
            if len(kept) < len(waits):
                ins.sync_info = mybir.SyncInfo(
                    on_wait=kept, on_update=list(si.on_update or []))
        else:
            for w in usable:
                a = achiever(w.id, w.wait_value)
                if a is not None and know[a]:
                    for s, v in know[a].items():
                        cur[s] = max(cur.get(s, 0), v)
                cur[w.id] = max(cur.get(w.id, 0), w.wait_value)
        if len(kept) > 1 and ins.__class__.__name__ in reducible:
            still_multi.append((ins.name, ins.__class__.__name__,
                                [(w.ant_name, w.wait_value) for w in kept]))
        if si:
            for up in (si.on_update or []):
                if getattr(up, "update_mode", None) in ("sem-inc",
                                                        "sem-add-imm"):
                    vals, idxs = sem_hist[up.id]
                    i = bisect.bisect_left(idxs, idx)
                    if i < len(idxs) and idxs[i] == idx:
                        cur[up.id] = max(cur.get(up.id, 0), vals[i])
        know[idx] = cur
        if proc is not None:
            last_on_proc[proc] = idx
    return still_multi


def _strip_waits(nc, mybir):
    """Drop every sem wait already implied by happens-before.

    Queues (procs) execute in order and sem values only increase, so a
    wait (sem >= v) is a runtime no-op whenever the waiting instruction's
    proc-predecessor chain (plus the waits that chain performed) already
    guarantees sem >= v. Returns #dropped."""
    import bisect
    from collections import defaultdict

    f = nc.m.functions[0]
    insts = [ins for blk in f.blocks for ins in blk.instructions]

    cum = defaultdict(int)
    sem_hist = defaultdict(lambda: ([], []))
    bad_sems = set()
    for idx, ins in enumerate(insts):
        si = ins.sync_info
        if not si:
            continue
        for up in (si.on_update or []):
            if getattr(up, "update_mode", None) in ("sem-inc", "sem-add-imm"):
                cum[up.id] += up.update_value
                vals, idxs = sem_hist[up.id]
                vals.append(cum[up.id])
                idxs.append(idx)
            else:
                bad_sems.add(up.id)

    def achiever(sem, v):
        if sem in bad_sems:
            return None
        vals, idxs = sem_hist.get(sem, ([], []))
        i = bisect.bisect_left(vals, v)
        return idxs[i] if i < len(vals) else None

    know = [None] * len(insts)
    last_on_proc = {}
    dropped = 0
    for idx, ins in enumerate(insts):
        proc = getattr(ins, "bass_scheduled_proc", None)
        cur = {}
        if proc is not None and proc in last_on_proc:
            cur = dict(know[last_on_proc[proc]])
        si = ins.sync_info
        waits = list(si.on_wait or []) if si else []
        kept = []
        for w in waits:
            usable = (getattr(w, "wait_mode", None) == "sem-ge-imm"
                      and w.wait_reg is None and w.id not in bad_sems)
            if usable and cur.get(w.id, 0) >= w.wait_value:
                dropped += 1
                continue
            kept.append(w)
            if usable:
                a = achiever(w.id, w.wait_value)
                if a is not None and know[a]:
                    for s, v in know[a].items():
                        cur[s] = max(cur.get(s, 0), v)
                cur[w.id] = max(cur.get(w.id, 0), w.wait_value)
        if si and len(kept) < len(waits):
            ins.sync_info = mybir.SyncInfo(
                on_wait=kept, on_update=list(si.on_update or []))
        if si:
            for up in (si.on_update or []):
                if getattr(up, "update_mode", None) in ("sem-inc",
                                                        "sem-add-imm"):
                    vals, idxs = sem_hist[up.id]
                    i = bisect.bisect_left(idxs, idx)
                    if i < len(idxs) and idxs[i] == idx:
                        cur[up.id] = max(cur.get(up.id, 0), vals[i])
        know[idx] = cur
        if proc is not None:
            last_on_proc[proc] = idx
    return dropped


QUAD = True


def _get_bass_quad(repeat=1, g=G, dtype=DTYPE):
    key = ("ncq", repeat, g, dtype)
    if key in _CACHE:
        return _CACHE[key]
    import sys
    for p in ("/opt/trn_rl_repo", "/root/.axon_site/_ro/trn_rl_repo"):
        if p not in sys.path:
            sys.path.append(p)
    import concourse.bass as bass
    import concourse.tile as tile
    from concourse import mybir

    values, taps, emitted, finals, col_vids, phase_of = _build_network_quad()

    # phase-local level-BFS reordering: the odd-even merges build long
    # RAW chains when emitted depth-first; consecutive dependent narrow
    # ops stall on the DVE's result->operand latency (measured ~2x on
    # the pairs/band phases).  Sorting by (phase, level) makes adjacent
    # instructions independent while keeping liveness phase-local.
    import os as _os
    lvl = {tp: 0 for tp in taps}
    for (alu, d, aref, bref, lo, hi) in emitted:
        lvl[d] = 1 + max(lvl[aref[0]], lvl[bref[0]])
    order = {e[1]: i for i, e in enumerate(emitted)}
    bfs_mode = _os.environ.get('QUAD_BFS', '0')
    BFS_PHASES = {2, 4, 5} if bfs_mode != '0' else set()

    def srt(e):
        ph = phase_of.get(e[1], 9)
        if bfs_mode in ('2', '3') and ph > 1:
            # global cross-phase level sort (phases 2-5 interleaved);
            # mode 3 additionally orders each level so consumers of the
            # OLDEST producers run first (maximizes RAW distance)
            age = (max(order.get(e[2][0], -1), order.get(e[3][0], -1))
                   if bfs_mode == '3' else 0)
            return (2, lvl[e[1]], age, order[e[1]])
        lv = lvl[e[1]] if ph in BFS_PHASES else 0
        return (ph, lv, 0, order[e[1]])
    emitted = sorted(emitted, key=srt)

    # fuse independent same-alu nv=1 (quad) ops into 2-lane instructions
    # (lane stride = plain address delta, affine for any region pair).
    # Window-lookahead on the original order keeps the liveness profile:
    # a partner op moves EARLIER to the fuse point, so it only needs its
    # inputs already produced there; min/max commute, so operands are
    # canonicalized to matching class order per lane.
    fin_set = set(finals.values())
    prod_pos = {}
    for i, e in enumerate(emitted):
        prod_pos[e[1]] = i

    def canon(e):
        alu, d, aref, bref, lo, hi = e
        ca, cb = values[aref[0]], values[bref[0]]
        if ca > cb:
            aref, bref, ca, cb = bref, aref, cb, ca
            e = (alu, d, aref, bref, lo, hi)
        return e, ca, cb

    # Fusing independent nv=1 ops into 2-lane instructions measured
    # SLOWER on hardware (~+17us/core, interleaved A/B) despite fewer
    # instructions -- arbitrary lane strides cost more than the saved
    # issue overhead.  Kept behind an env flag, default off.
    import os
    WINDOW = int(os.environ.get('QUAD_PAIR_WINDOW', '0'))
    used = set()
    new_emitted = []
    pair_with = {}
    for i, e in enumerate(emitted):
        if id(e) in used:
            continue
        if values[e[1]] != 1 or e[1] in fin_set:
            new_emitted.append(e)
            continue
        e1, ca1, cb1 = canon(e)
        partner = None
        for j in range(i + 1, min(i + 1 + WINDOW, len(emitted))):
            e2 = emitted[j]
            if (id(e2) in used or values[e2[1]] != 1
                    or e2[1] in fin_set or e2[0] != e1[0]):
                continue
            e2c, ca2, cb2 = canon(e2)
            if (ca2, cb2) != (ca1, cb1):
                continue
            # partner's inputs must exist at the fuse point
            if all(prod_pos.get(r[0], -1) < i for r in (e2c[2], e2c[3])):
                partner = e2c
                used.add(id(e2))
                break
        new_emitted.append(e1)
        if partner is not None:
            pair_with[id(e1)] = partner
    emitted = new_emitted
    skip = {id(e2) for e2 in pair_with.values()}
    # keep fused partners adjacent in the allocator's view
    alloc_order = []
    fused_first = set()
    for e in emitted:
        if id(e) in pair_with:
            fused_first.add(len(alloc_order))
            alloc_order.append(e)
            alloc_order.append(pair_with[id(e)])
        else:
            alloc_order.append(e)
    import os as _os
    max_phase = int(_os.environ.get('QUAD_MAX_PHASE', '5'))
    if max_phase < 5:
        keep = [e for e in emitted if phase_of.get(e[1], 9) <= max_phase]
        emitted = keep
        alloc_order = [e for e in alloc_order
                       if phase_of.get(e[1], 9) <= max_phase]
    region, nreg = _arena_alloc_quad(values, taps, alloc_order, finals,
                                     fused_first)
    fin_r = {v: r for r, v in finals.items()}
    tapset = set(taps)
    import os as _os0
    NWAVES = int(_os0.environ.get('QUAD_WAVES', '2'))
    tap_plane = {tp: (k if (NWAVES == 1 or k < 4) else k - 4)
                 for k, tp in enumerate(taps)}
    R_OF_SUB = {s: r for r, s in SUB_OF_R.items()}

    RS = {c: g * c * SP_Q for c in (4, 2, 1)}       # region sizes
    zb = {4: 0}
    zb[2] = nreg[4] * RS[4]
    zb[1] = zb[2] + nreg[2] * RS[2]
    zstage = zb[1] + nreg[1] * RS[1]    # staging planes (pitch-padded:
    STG_P = W + 8                       # 4 zero cols each side feed the
    nplanes = 4 if NWAVES == 2 else K   # sort edge slots as padding
    NEL = zstage + nplanes * g * STG_P

    def vaddr(vid, sub, s):
        nv = values[vid]
        return zb[nv] + region[vid] * RS[nv] + sub * SP_Q + s

    dt = mybir.dt.bfloat16 if dtype == "bf16" else mybir.dt.float32
    nc = bass.Bass("TRN2", target_bir_lowering=False, debug=False)
    img_pad = nc.dram_tensor("img_pad", [C, H + 2 * PAD, W], dt,
                             kind="ExternalInput").ap()
    out = nc.dram_tensor("out", [C * H, W], dt, kind="ExternalOutput").ap()

    assert NUNITS % g == 0
    units = [(c, b) for c in range(C) for b in range(H // 128)]
    groups = [units[i:i + g] for i in range(0, NUNITS, g)] * repeat

    with tile.TileContext(nc) as tc:
        with tc.tile_pool(name="mem", bufs=1) as pool:
            arena = pool.tile([128, NEL], dt, tag="arena", name="arena")
            ft = pool.tile([128, g * W], dt, tag="ft", name="ft")
            scr = pool.tile([128, 1], dt, tag="scr", name="scr")

            def ap(tile_, el_off, dims):
                b = tile_[:, 0:1]
                return bass.AP(tensor=b.tensor, offset=b.offset + el_off,
                               ap=[[tile_.shape[1], 128]] + dims)

            for gi, grp in enumerate(groups):
                # taps in 2 DMA waves into 4 staging planes (rows 0-3,
                # then rows 4-6 reusing planes 0-2).  The permuted sort
                # inputs make records 0..3 (CE(w0,w6), CE(w2,w3)) read
                # wave-A taps only; they are emitted BETWEEN the waves so
                # their reads bind to wave-A data before planes 0-2 are
                # overwritten.
                def tap_wave(tap0, ntap):
                    for gslot, (c, b) in enumerate(grp):
                        src = img_pad[c, 128 * b + tap0:
                                      128 * b + tap0 + 1, 0:W]
                        srcap = bass.AP(
                            tensor=src.tensor, offset=src.offset,
                            ap=[[W, 128], [W, ntap], [1, W]])
                        dst = ap(arena, zstage + gslot * STG_P + 4,
                                 [[g * STG_P, ntap], [1, W]])
                        nc.sync.dma_start(out=dst, in_=srcap)
                    for gslot in range(len(grp)):
                        nc.vector.tensor_copy(
                            scr[:, :],
                            arena[:, zstage + gslot * STG_P + 4:
                                  zstage + gslot * STG_P + 5])
                        tc.no_sync_barrier()

                if gi == 0:
                    # zero the staging pad columns ONCE: the DMAs only
                    # ever write the interior, so they stay zero
                    for off in (0, STG_P - 4):
                        nc.vector.memset(
                            ap(arena, zstage + off,
                               [[STG_P, 4 * g], [1, 4]]), 0.0)
                if NWAVES == 1:
                    tap_wave(0, K)
                else:
                    tap_wave(0, 4)
                n_sort = sum(1 for e in emitted if values[e[1]] == 4)
                for ei, (alu, d, aref, bref, lo, hi) in enumerate(emitted):
                    if ei == 4 and NWAVES == 2:
                        tap_wave(4, 3)
                    nv = values[d]
                    n = hi - lo + 1
                    op = (mybir.AluOpType.min if alu == "min"
                          else mybir.AluOpType.max)
                    touches_tap = aref[0] in tapset or bref[0] in tapset

                    def src_ap(ref, l0, nl, nn):
                        # lanes l0 .. l0+nl-1 of ref as a 4-dim AP
                        sv, sub0, dsub, q0, dq = ref
                        base = vaddr(sv, sub0 + l0 * dsub,
                                     lo + q0 + l0 * dq)
                        dims = [[values[sv] * SP_Q, g]]
                        if nl > 1:
                            dims.append([dsub * SP_Q + dq, nl])
                        dims.append([1, nn])
                        return ap(arena, base, dims)

                    def src_ap_tap(ref, l0, nl, nn):
                        # sub-lane l reads raw staging: variant
                        # r = R_OF_SUB[l], column c = 4*(s-1)+r at
                        # offset 4*s + r - 4; R_OF_SUB steps by -2
                        # between lane pairs (0,1) and (2,3).
                        sv = ref[0]
                        base = (zstage + tap_plane[sv] * g * STG_P
                                + 4 * lo + R_OF_SUB[l0])
                        dims = [[STG_P, g]]
                        if nl > 1:
                            dims.append([-2, nl])
                        dims.append([4, nn])
                        return ap(arena, base, dims)

                    span_mode = _os.environ.get('QUAD_SPAN', '0') == '1'
                    contig = all(dsub == 1 and dq == 0
                                 for (_, _, dsub, _, dq) in (aref, bref))
                    if touches_tap:
                        # two paired 2-lane instructions (lanes 0,1 / 2,3)
                        for h in (0, 2):
                            dstap = ap(arena, vaddr(d, h, lo),
                                       [[nv * SP_Q, g], [SP_Q, 2], [1, n]])
                            ins = [(src_ap_tap(ref, h, 2, n)
                                    if ref[0] in tapset
                                    else src_ap(ref, h, 2, n))
                                   for ref in (aref, bref)]
                            nc.vector.tensor_tensor(
                                out=dstap, in0=ins[0], in1=ins[1], op=op)
                    elif d in fin_r:
                        pr = fin_r[d]   # pair index: ft x = 4q + pr + 2l
                        dstap = ap(ft, 4 * (lo - 1) + pr,
                                   [[W, g], [2, 2], [4, n]])
                        nc.vector.tensor_tensor(
                            out=dstap, in0=src_ap(aref, 0, 2, n),
                            in1=src_ap(bref, 0, 2, n), op=op)
                    elif nv == 1:
                        e2 = pair_with.get(id(emitted[ei]))
                        if e2 is not None:
                            # fused 2-lane op: lane 1 is the partner op;
                            # lane strides are plain address deltas
                            _, d2, aref2, bref2, lo2, hi2 = e2
                            ulo, uhi = min(lo, lo2), max(hi, hi2)
                            un = uhi - ulo + 1

                            def fap(r1, r2):
                                sv1, sub1, _, q1, _ = r1
                                sv2, sub2, _, q2, _ = r2
                                b1 = vaddr(sv1, sub1, ulo + q1)
                                b2 = vaddr(sv2, sub2, ulo + q2)
                                return ap(arena, b1,
                                          [[values[sv1] * SP_Q, g],
                                           [b2 - b1, 2], [1, un]])
                            db1 = vaddr(d, 0, ulo)
                            db2 = vaddr(d2, 0, ulo)
                            dstap = ap(arena, db1,
                                       [[SP_Q, g], [db2 - db1, 2],
                                        [1, un]])
                            nc.vector.tensor_tensor(
                                out=dstap, in0=fap(aref, aref2),
                                in1=fap(bref, bref2), op=op)
                        else:
                            dstap = ap(arena, vaddr(d, 0, lo),
                                       [[SP_Q, g], [1, n]])
                            nc.vector.tensor_tensor(
                                out=dstap, in0=src_ap(aref, 0, 1, n),
                                in1=src_ap(bref, 0, 1, n), op=op)
                    elif span_mode and contig:
                        # 3-dim contiguous span across lanes: processes
                        # (nv-1)*SP+n elements instead of nv*n, but with
                        # one less AP dim
                        span = (nv - 1) * SP_Q + n

                        def span_ap(ref):
                            sv, sub0, dsub, q0, dq = ref
                            base = vaddr(sv, sub0, lo + q0)
                            return ap(arena, base,
                                      [[values[sv] * SP_Q, g], [1, span]])
                        dstap = ap(arena, vaddr(d, 0, lo),
                                   [[nv * SP_Q, g], [1, span]])
                        nc.vector.tensor_tensor(
                            out=dstap, in0=span_ap(aref),
                            in1=span_ap(bref), op=op)
                    else:
                        dstap = ap(arena, vaddr(d, 0, lo),
                                   [[nv * SP_Q, g], [SP_Q, nv], [1, n]])
                        nc.vector.tensor_tensor(
                            out=dstap, in0=src_ap(aref, 0, nv, n),
                            in1=src_ap(bref, 0, nv, n), op=op)


                c0, bb0 = grp[0]
                row0 = (c0 * (H // 128) + bb0) * 128
                ob = out[row0:row0 + 1, 0:W]
                odst = bass.AP(tensor=ob.tensor, offset=ob.offset,
                               ap=[[W, 128], [128 * W, g], [1, W]])
                nc.sync.dma_start(out=odst,
                                  in_=ap(ft, 0, [[W, g], [1, W]]))

            for _ in range(2):
                nc.vector.memset(ft[:, 0:1], 0.0)
                tc.no_sync_barrier()

    leftover = _reduce_waits(nc, mybir)
    if max_phase >= 5:
        assert not leftover, f"multi-wait instructions remain: {leftover[:5]}"
    elif leftover:
        # timing-only truncated build: keep one wait per instruction
        f = nc.m.functions[0]
        for blk in f.blocks:
            for ins in blk.instructions:
                si = ins.sync_info
                if si and si.on_wait and len(si.on_wait) > 1:
                    ins.sync_info = mybir.SyncInfo(
                        on_wait=[si.on_wait[0]],
                        on_update=list(si.on_update or []))
    _strip_waits(nc, mybir)
    _CACHE[key] = nc
    return nc


def _get_bass(repeat=1, g=G, dtype=DTYPE):
    if QUAD:
        return _get_bass_quad(repeat, g, dtype)
    key = ("nc", repeat, g, dtype)
    if key in _CACHE:
        return _CACHE[key]
    import sys
    for p in ("/opt/trn_rl_repo", "/root/.axon_site/_ro/trn_rl_repo"):
        if p not in sys.path:
            sys.path.append(p)
    import concourse.bass as bass
    import concourse.tile as tile
    from concourse import mybir

    emitted, final_id = _build_network()
    region, n_regions = _arena_alloc(emitted, final_id)
    assert n_regions <= 30, n_regions

    dt = mybir.dt.bfloat16 if dtype == "bf16" else mybir.dt.float32
    nc = bass.Bass("TRN2", target_bir_lowering=False, debug=False)
    # img arrives zero-row-padded per channel: [C, H+6, W]
    img_pad = nc.dram_tensor("img_pad", [C, H + 2 * PAD, W], dt,
                             kind="ExternalInput").ap()
    # out viewed as [C*H, W]: 12 row-blocks of 128 with uniform 128*W pitch
    out = nc.dram_tensor("out", [C * H, W], dt, kind="ExternalOutput").ap()

    assert NUNITS % g == 0
    units = [(c, b) for c in range(C) for b in range(H // 128)]
    groups = [units[i:i + g] for i in range(0, NUNITS, g)] * repeat

    R = g * PITCH                    # arena region size in elements
    NEL = n_regions * R              # arena free-dim elements
    wid_last = PITCH * (g - 1)

    with tile.TileContext(nc) as tc:
        with tc.tile_pool(name="mem", bufs=1) as pool:
            arena = pool.tile([128, NEL], dt, tag="arena", name="arena")
            ft = pool.tile([128, R], dt, tag="ft", name="ft")
            scr = pool.tile([128, 1], dt, tag="scr", name="scr")

            def ap3(tile_, el_off, d1, n1, d2, n2):
                b = tile_[:, 0:1]
                return bass.AP(tensor=b.tensor, offset=b.offset + el_off,
                               ap=[[tile_.shape[1], 128], [d1, n1], [d2, n2]])

            for gi, grp in enumerate(groups):
                # re-zero the tap pad columns (slot tenants clobber them
                # between groups): cols [0:3] and [515:518] per (tap, block)
                b0 = arena[:, 0:1]
                for off in (0, W + PAD):
                    nc.vector.memset(
                        bass.AP(tensor=b0.tensor, offset=b0.offset + off,
                                ap=[[NEL, 128], [R, K], [PITCH, g],
                                    [1, PAD]]),
                        0.0)
                # vertical taps: one strided DMA per (channel,row-block)
                # slice; tap k of slice gslot lands at arena element
                # k*R + gslot*PITCH + PAD + x
                for gslot, (c, b) in enumerate(grp):
                    src = img_pad[c, 128 * b:128 * b + 1, 0:W]
                    srcap = bass.AP(tensor=src.tensor, offset=src.offset,
                                    ap=[[W, 128], [W, K], [1, W]])
                    nc.sync.dma_start(
                        out=ap3(arena, gslot * PITCH + PAD, R, K, 1, W),
                        in_=srcap)
                # fan-in: one tiny copy per slice absorbs its DMA-queue
                # wait (1-wait ISA limit)
                for gslot in range(g):
                    nc.vector.tensor_copy(
                        scr[:, :],
                        arena[:, gslot * PITCH + PAD:gslot * PITCH + PAD + 1])
                    tc.no_sync_barrier()

                for (o, s, (sa, da), (sb, db), lo, hi) in emitted:
                    n = wid_last + hi - lo + 1

                    def src_ap(v, d):
                        base = ((-1 - v) if v < 0 else region[v]) * R
                        return arena[:, base + lo + d:base + lo + d + n]

                    ta = src_ap(sa, da)
                    tb = src_ap(sb, db)
                    if s == final_id:
                        tdst = ft[:, lo:lo + n]
                    else:
                        base = region[s] * R
                        tdst = arena[:, base + lo:base + lo + n]
                    op = (mybir.AluOpType.min if o == "min"
                          else mybir.AluOpType.max)
                    nc.vector.tensor_tensor(out=tdst, in0=ta, in1=tb, op=op)

                # one out-DMA per group; units are lex-consecutive so the
                # dst block pitch is uniformly 128*W in the [C*H, W] view
                c0, bb0 = grp[0]
                row0 = (c0 * (H // 128) + bb0) * 128
                ob = out[row0:row0 + 1, 0:W]
                odst = bass.AP(tensor=ob.tensor, offset=ob.offset,
                               ap=[[W, 128], [128 * W, g], [1, W]])
                nc.sync.dma_start(out=odst, in_=ap3(ft, 0, PITCH, g, 1, W))

            # tail ladder: WAR-touch ft so the DVE chain observes the out-
            # DMA queue completions (framework drain then needs <=1 wait)
            for _ in range(2):
                nc.vector.memset(ft[:, 0:1], 0.0)
                tc.no_sync_barrier()

    leftover = _reduce_waits(nc, mybir)
    assert not leftover, f"multi-wait instructions remain: {leftover[:5]}"
    _strip_waits(nc, mybir)
    _CACHE[key] = nc
    return nc


def _to_dev_dtype(x, dtype):
    if dtype == "f32":
        return np.ascontiguousarray(x, dtype=np.float32)
    import ml_dtypes
    return np.ascontiguousarray(x.astype(ml_dtypes.bfloat16))


_RUNNERS = {}


def _get_runner(nc, n_cores=8):
    """Compile nc's 8-core SPMD executable ONCE and return a reusable
    callable. bass2jax.run_bass_via_pjrt rebuilds the jit closure every
    call (full retrace + XLA compile with the BIR embedded in the HLO),
    so per-call wall time scales with program size; caching the jitted
    callable leaves only dispatch + device execution per call."""
    key = id(nc)
    if key in _RUNNERS:
        return _RUNNERS[key]
    import jax
    import numpy as _np
    from jax.sharding import Mesh, PartitionSpec
    from jax.experimental.shard_map import shard_map
    from concourse import mybir
    from concourse import bass2jax
    bass2jax.install_neuronx_cc_hook()

    partition_name = (nc.partition_id_tensor.name
                      if nc.partition_id_tensor else None)
    in_names = []
    out_names = []
    out_avals = []
    zero_shapes = []
    for alloc in nc.m.functions[0].allocations:
        if not isinstance(alloc, mybir.MemoryLocationSet):
            continue
        name = alloc.memorylocations[0].name
        if alloc.kind == "ExternalInput":
            if name != partition_name:
                in_names.append(name)
        elif alloc.kind == "ExternalOutput":
            shape = tuple(alloc.tensor_shape)
            dtype = mybir.dt.np(alloc.dtype)
            out_names.append(name)
            out_avals.append(jax.core.ShapedArray(shape, dtype))
            zero_shapes.append((shape, dtype))
    n_params = len(in_names)
    n_outs = len(out_avals)
    all_names = list(in_names) + list(out_names)
    if partition_name is not None:
        all_names.append(partition_name)
    donate = tuple(range(n_params, n_params + n_outs))

    def _body(*args):
        operands = list(args)
        if partition_name is not None:
            operands.append(bass2jax.partition_id_tensor())
        outs = bass2jax._bass_exec_p.bind(
            *operands,
            out_avals=tuple(out_avals),
            in_names=tuple(all_names),
            out_names=tuple(out_names),
            lowering_input_output_aliases=(),
            sim_require_finite=True,
            sim_require_nnan=True,
            nc=nc,
        )
        return tuple(outs)

    devices = jax.devices()[:n_cores]
    assert len(devices) == n_cores
    mesh = Mesh(_np.asarray(devices), ("core",))
    in_specs = (PartitionSpec("core"),) * (n_params + n_outs)
    out_specs = (PartitionSpec("core"),) * n_outs
    sharded = jax.jit(
        shard_map(_body, mesh=mesh, in_specs=in_specs, out_specs=out_specs,
                  check_rep=False),
        donate_argnums=donate, keep_unused=True)

    def run(in_maps, raw=False):
        concat_in = [
            _np.concatenate([_np.asarray(in_maps[c][nm])
                             for c in range(n_cores)], axis=0)
            for nm in in_names]
        concat_zeros = [_np.zeros((n_cores * s[0], *s[1:]), d)
                        for (s, d) in zero_shapes]
        out_arrs = sharded(*concat_in, *concat_zeros)
        if raw:
            return out_arrs
        return [
            {nm: _np.asarray(out_arrs[i]).reshape(n_cores,
                                                  *out_avals[i].shape)[c]
             for i, nm in enumerate(out_names)}
            for c in range(n_cores)]

    def run_timed(in_maps, reps):
        """Pre-stage inputs on device, then time exec-only calls."""
        import time as _time
        import jax.numpy as jnp
        concat_in = [
            _np.concatenate([_np.asarray(in_maps[c][nm])
                             for c in range(n_cores)], axis=0)
            for nm in in_names]
        staged = jax.device_put(concat_in)
        jax.block_until_ready(staged)
        ts = []
        for _ in range(reps + 1):
            # on-device zeros: no H2D transfer in the timed region
            concat_zeros = [jnp.zeros((n_cores * s[0], *s[1:]), d)
                            for (s, d) in zero_shapes]
            jax.block_until_ready(concat_zeros)
            t0 = _time.time()
            out = sharded(*staged, *concat_zeros)
            jax.block_until_ready(out)
            ts.append(_time.time() - t0)
        return min(ts[1:])  # first is warmup (compile)

    run.timed = run_timed
    _RUNNERS[key] = run
    return run


def kernel(img: np.ndarray) -> np.ndarray:
    import sys
    for p in ("/opt/trn_rl_repo", "/root/.axon_site/_ro/trn_rl_repo"):
        if p not in sys.path:
            sys.path.append(p)

    img = np.asarray(img, dtype=np.float32)
    assert img.shape == (B, C, H, W), img.shape
    nc = _get_bass()
    padded = np.pad(img, ((0, 0), (0, 0), (PAD, PAD), (0, 0)))
    in_maps = [{"img_pad": _to_dev_dtype(padded[b], DTYPE)}
               for b in range(B)]
    results = _get_runner(nc)(in_maps)
    outs = [np.asarray(results[b]["out"]).astype(np.float32)
            .reshape(C, H, W) for b in range(B)]
    return np.stack(outs, axis=0)

